# revision 6
# baseline (speedup 1.0000x reference)
"""Per-domain batch normalization (BaseDomainBatchNorm) on 8 Trainium2 NeuronCores.

Math (reference):
    cnt[j]   = #{n : d[n] == j}            (clamped to >= 1)
    mean[j]  = sum_{d[n]==j} X[n] / cnt[j]
    var[j]   = sum_{d[n]==j} X[n]^2 / cnt[j] - mean[j]^2
    inv[j]   = rsqrt(var[j] + 1e-5)
    Y[n]     = (X[n] - mean[d[n]]) * inv[d[n]] * gamma[d[n]] + beta[d[n]]
             = X[n] * A[d[n]] + B[d[n]],  A = inv*gamma, B = beta - mean*A

Sharding: rows (samples) split 8192 per core; per-domain partial stats
(sum / sumsq / count) are AllReduce'd across the 8 cores; each core then
normalizes its own rows.  gamma/beta replicated.

Per-core pipeline (64 chunks of 128 rows; chunk i, partition p = row p*64+i):
  phase 1:  X arrives in 1 MiB group DMAs -> DVE casts to a resident bf16
            copy xb; squares split DVE/ACT; one-hot stats matmuls accumulate
            sum/sumsq in PSUM (bf16 data, fp32 accumulate).
  AR:       [16, 1025] partial stats AllReduce'd via collective_compute.
  finalize: A = gamma*rsqrt(var+eps) via ACT Abs_reciprocal_sqrt;
            B = beta - mean*A; tables stored bf16 at
            partitions 0:16 (A) and 32:48 (B).
  phase 2:  per chunk: PE gathers A-rows and B-rows from the bf16 tables
            (one-hot matmuls; bf16 table entries gather EXACTLY);
            ACT casts the A-gather PSUM->SBUF bf16 (lossless);
            DVE computes m = xb * a_rows (bf16, 2x mode);
            PE adds m into the B-gather PSUM bank via an identity matmul;
            DVE evacuates Y fp32; 1 MiB group DMAs store Y.
HBM traffic is the roofline minimum: read X once, write Y once.
"""

import numpy as np

N = 65536
C = 512
D = 16
NCORES = 8
SHARD = N // NCORES          # 8192 rows per core
P = 128                      # partitions
CHUNKS = SHARD // P          # 64 chunks of 128 rows
GRP = 4                      # chunks per X/Y DMA group (1 MiB fp32)
NGRP = CHUNKS // GRP         # 16 groups
EPS = 1e-5
STAG = 3                     # phase-2 software pipeline stagger (chunks)

_CACHE = {}


def _build_program():
    import concourse.bacc as bacc
    import concourse.bass as bass
    import concourse.tile as tile
    from concourse import mybir

    f32 = mybir.dt.float32
    bf16 = mybir.dt.bfloat16
    i32 = mybir.dt.int32
    Alu = mybir.AluOpType
    Act = mybir.ActivationFunctionType

    nc = bacc.Bacc("TRN2", target_bir_lowering=False, debug=False,
                   num_devices=NCORES)

    X_d = nc.dram_tensor("X", [SHARD, C], f32, kind="ExternalInput")
    d_d = nc.dram_tensor("d", [SHARD], i32, kind="ExternalInput")
    g_d = nc.dram_tensor("gamma", [D, C], f32, kind="ExternalInput")
    b_d = nc.dram_tensor("beta", [D, C], f32, kind="ExternalInput")
    Y_d = nc.dram_tensor("Y", [SHARD, C], f32, kind="ExternalOutput")

    cc_in = nc.dram_tensor("cc_in", [D, 2 * C + 1], f32)
    cc_out = nc.dram_tensor("cc_out", [D, 2 * C + 1], f32, addr_space="Shared")

    # partition p owns rows [p*64, (p+1)*64): per-partition contiguous DMA
    Xv = X_d.ap().rearrange("(p n) c -> p n c", p=P)   # [128, 64, 512]
    Yv = Y_d.ap().rearrange("(p n) c -> p n c", p=P)

    DB = 2048  # d-broadcast strip width for the transposed one-hot build

    with tile.TileContext(nc) as tc:
        with (
            tc.tile_pool(name="const", bufs=1) as cpool,
            tc.tile_pool(name="xg", bufs=3) as xgpool,
            tc.tile_pool(name="xb", bufs=1) as xbpool,
            tc.tile_pool(name="sq", bufs=2) as sqpool,
            tc.tile_pool(name="oh", bufs=1) as ohpool,
            tc.tile_pool(name="dbc", bufs=2) as dbcpool,
            tc.tile_pool(name="small", bufs=1) as spool,
            tc.tile_pool(name="scr", bufs=2) as scrpool,
            tc.tile_pool(name="pab", bufs=3) as pabpool,
            tc.tile_pool(name="m", bufs=STAG + 2) as mpool,
            tc.tile_pool(name="y", bufs=3) as ypool,
        ):
            # ---- X load: issue the big input DMAs first ----
            xgs = []
            for g in range(NGRP):
                xg = xgpool.tile([P, GRP, C], f32)
                xgs.append(xg)
                nc.sync.dma_start(xg[:], Xv[:, GRP * g:GRP * (g + 1), :])

            # ---- constants ----
            # iota_rep[p, i, j] = j  (for the chunk-layout one-hot)
            iota_rep = cpool.tile([P, CHUNKS, D], bf16)
            nc.gpsimd.iota(iota_rep[:], pattern=[[0, CHUNKS], [1, D]], base=0,
                           channel_multiplier=0,
                           allow_small_or_imprecise_dtypes=True)
            # iota_col64[p, 0] = p % 16 as f32 (for the transposed one-hot)
            iota_i = cpool.tile([4 * D, 1], i32)
            nc.gpsimd.iota(iota_i[:], pattern=[[0, 1]], base=0,
                           channel_multiplier=1)
            nc.vector.tensor_scalar(iota_i[:], iota_i[:], D - 1, None,
                                    Alu.bitwise_and)
            iota_col64 = cpool.tile([4 * D, 1], f32)
            nc.vector.tensor_copy(iota_col64[:], iota_i[:])
            ones_col = cpool.tile([P, 1], bf16)
            nc.vector.memset(ones_col[:], 1.0)
            # identity (bf16) for the phase-2 add-via-matmul
            irow = cpool.tile([P, P], i32)
            nc.gpsimd.iota(irow[:], pattern=[[1, P]], base=0,
                           channel_multiplier=0)
            icol = cpool.tile([P, 1], i32)
            nc.gpsimd.iota(icol[:], pattern=[[0, 1]], base=0,
                           channel_multiplier=1)
            icolf = cpool.tile([P, 1], f32)
            nc.vector.tensor_copy(icolf[:], icol[:])
            ident = cpool.tile([P, P], bf16)
            nc.vector.tensor_scalar(ident[:], irow[:], icolf[:], None,
                                    Alu.is_equal)

            # ---- d in chunk layout ([p, n]) and one-hot [128, 64, 16] ----
            d_pn = cpool.tile([P, CHUNKS], i32)
            nc.sync.dma_start(d_pn[:], d_d.ap().rearrange("(p n) -> p n", p=P))
            d_f = cpool.tile([P, CHUNKS], bf16)
            nc.vector.tensor_copy(d_f[:], d_pn[:])
            onehot = ohpool.tile([P, CHUNKS, D], bf16)
            nc.vector.tensor_tensor(
                onehot[:], iota_rep[:],
                d_f[:].unsqueeze(-1).broadcast_to([P, CHUNKS, D]),
                Alu.is_equal)

            # ---- transposed one-hot on partitions 0:64 ----
            # rows 0:16 and 32:48 both hold onehot(d) (p%16 iota); rows
            # 16:32 / 48:64 are identical copies that are simply unused.
            # Row n = p*64 + i (natural shard order).
            ohT = ohpool.tile([4 * D, SHARD], bf16)
            for h in range(SHARD // DB):
                d_bc = dbcpool.tile([4 * D, DB], i32)
                src = d_d.ap()[h * DB:(h + 1) * DB]
                src = src.rearrange("(a n) -> a n", a=1).partition_broadcast(4 * D)
                nc.gpsimd.dma_start(d_bc[:], src)
                nc.vector.tensor_scalar(ohT[:, h * DB:(h + 1) * DB],
                                        d_bc[:], iota_col64[:], None,
                                        Alu.is_equal)

            # counts: reduce one-hot over chunks (independent of X; early)
            rowcnt = spool.tile([P, D], f32, tag="rowcnt")
            nc.vector.tensor_reduce(
                rowcnt[:], onehot[:].rearrange("p n d -> p d n"),
                mybir.AxisListType.X, Alu.add)
            rowcnt_bf = spool.tile([P, D], bf16, tag="rowcnt_bf")
            nc.vector.tensor_copy(rowcnt_bf[:], rowcnt[:])

            # gamma/beta prefetch
            gam = spool.tile([D, C], f32, tag="gam")
            nc.sync.dma_start(gam[:], g_d[:])
            bet = spool.tile([D, C], f32, tag="bet")
            nc.sync.dma_start(bet[:], b_d[:])

            # resident bf16 copy of X: [128, 64 chunks, 512]
            xball = xbpool.tile([P, CHUNKS, C], bf16)

            # ---- phase 1: per-core partial stats ----
            stats = spool.tile([D, 2 * C + 1], f32, tag="stats")
            with tc.tile_pool(name="ps1", bufs=1, space="PSUM") as ps1:
                psum_s = ps1.tile([D, C], f32)
                psum_q = ps1.tile([D, C], f32)
                psum_c = ps1.tile([D, 1], f32)
                for g in range(NGRP):
                    xg = xgs[g]
                    xgf = xg[:].rearrange("p n c -> p (n c)")
                    xbg = xball[:, GRP * g:GRP * (g + 1), :] \
                        .rearrange("p n c -> p (n c)")
                    nc.vector.tensor_copy(xbg, xgf)
                    xsq = sqpool.tile([P, GRP * C], bf16, tag="xsq")
                    if g % 2 == 0:
                        nc.scalar.activation(xsq[:], xbg, Act.Square)
                    else:
                        nc.vector.tensor_mul(xsq[:], xbg, xbg)
                    for k in range(GRP):
                        i = GRP * g + k
                        oh = onehot[:, i, :]
                        st, sp = (i == 0), (i == CHUNKS - 1)
                        nc.tensor.matmul(psum_s[:], oh,
                                         xball[:, i, :],
                                         start=st, stop=sp)
                        nc.tensor.matmul(psum_q[:], oh,
                                         xsq[:, k * C:(k + 1) * C],
                                         start=st, stop=sp)

                nc.tensor.matmul(psum_c[:], rowcnt_bf[:], ones_col[:],
                                 start=True, stop=True)

                # ---- copy stats out of PSUM before freeing it ----
                nc.vector.tensor_copy(stats[:, 0:C], psum_s[:])
                nc.vector.tensor_copy(stats[:, C:2 * C], psum_q[:])
                nc.vector.tensor_copy(stats[:, 2 * C:2 * C + 1], psum_c[:])

                # a few matmuls to keep the PE HAM clock-gate warm a bit
                # into the all-reduce stall
                warm = ps1.tile([P, C], f32)
                for _ in range(8):
                    nc.tensor.matmul(warm[:], ident[0:P // 2, :],
                                     ohT[0:P // 2, 0:C],
                                     start=True, stop=True,
                                     skip_group_check=True)

            # ---- all-reduce partial stats across the 8 cores ----
            nc.sync.dma_start(cc_in[:], stats[:])
            nc.gpsimd.collective_compute(
                "AllReduce", Alu.add,
                replica_groups=[list(range(NCORES))],
                ins=[cc_in[:]], outs=[cc_out[:]])
            red = spool.tile([D, 2 * C + 1], f32, tag="red")
            nc.sync.dma_start(red[:], cc_out[:])

            # ---- finalize: A = inv*gamma, B = beta - mean*A ----
            cntc = spool.tile([D, 1], f32, tag="cntc")
            nc.vector.tensor_scalar_max(cntc[:], red[:, 2 * C:2 * C + 1], 1.0)
            rinv = spool.tile([D, 1], f32, tag="rinv")
            nc.vector.reciprocal(rinv[:], cntc[:])
            # mean | E[x^2] in one strip: [16, 1024]
            mm = spool.tile([D, 2 * C], f32, tag="mm")
            nc.vector.tensor_scalar_mul(mm[:], red[:, 0:2 * C], rinv[:])
            mean = mm[:, 0:C]
            var = spool.tile([D, C], f32, tag="var")
            nc.vector.scalar_tensor_tensor(var[:], mean, -1.0, mean,
                                           Alu.mult, Alu.mult)   # -mean^2
            nc.vector.tensor_add(var[:], var[:], mm[:, C:2 * C])
            epsb = spool.tile([D, 1], f32, tag="epsb")
            nc.vector.memset(epsb[:], EPS)
            # inv = rsqrt(var + eps)  (var+eps > 0, so |.| is a no-op)
            inv = spool.tile([D, C], f32, tag="inv")
            nc.scalar.activation(inv[:], var[:], Act.Abs_reciprocal_sqrt,
                                 bias=epsb[:])
            a_t = spool.tile([D, C], f32, tag="a_t")
            nc.vector.tensor_mul(a_t[:], inv[:], gam[:])
            negma = scrpool.tile([D, C], f32, tag="scr")
            nc.vector.scalar_tensor_tensor(negma[:], mean, -1.0, a_t[:],
                                           Alu.mult, Alu.mult)   # -mean*A
            b_t = spool.tile([D, C], f32, tag="b_t")
            nc.vector.tensor_add(b_t[:], bet[:], negma[:])

            # bf16 tables: A at partitions 0:16, B at partitions 32:48 so
            # each pairs with its own copy of the one-hot rows in ohT.
            tabs = spool.tile([4 * D, C], bf16, tag="tabs")
            nc.vector.tensor_copy(tabs[0:D, :], a_t[:])
            nc.scalar.activation(tabs[2 * D:3 * D, :], b_t[:], Act.Copy)

            ohTv = ohT[:].rearrange("k (p i) -> k i p", i=CHUNKS)

            # ---- phase 2: gather A/B per row, Y = X*A + B ----
            with tc.tile_pool(name="ps2a", bufs=3, space="PSUM") as ps2a, \
                 tc.tile_pool(name="ps2b", bufs=3, space="PSUM") as ps2b:
                psa = [None] * CHUNKS
                psb = [None] * CHUNKS
                mt = [None] * CHUNKS
                yg = None

                def tail(i):
                    # add m into the B-gather bank, then gather B on top
                    nonlocal yg
                    psb[i] = ps2b.tile([P, C], f32, tag="psb", name="psb")
                    nc.tensor.matmul(psb[i][:], ident[:], mt[i][:],
                                     start=True, stop=False)
                    nc.tensor.matmul(psb[i][:], ohTv[2 * D:3 * D, i, :],
                                     tabs[2 * D:3 * D, :],
                                     start=False, stop=True)
                    g, k = divmod(i, GRP)
                    if k == 0:
                        yg = ypool.tile([P, GRP, C], f32, tag="yg", name="yg")
                    nc.vector.tensor_copy(yg[:, k, :], psb[i][:])
                    if k == GRP - 1:
                        nc.sync.dma_start(Yv[:, GRP * g:GRP * (g + 1), :],
                                          yg[:])

                for i in range(CHUNKS):
                    psa[i] = ps2a.tile([P, C], f32, tag="psa", name="psa")
                    nc.tensor.matmul(psa[i][:], ohTv[0:D, i, :], tabs[0:D, :],
                                     start=True, stop=True)
                    if i >= STAG:
                        tail(i - STAG)
                    pab = pabpool.tile([P, C], bf16)
                    nc.scalar.activation(pab[:], psa[i][:], Act.Copy)
                    mt[i] = mpool.tile([P, C], bf16, tag="mt", name="mt")
                    nc.vector.tensor_mul(mt[i][:], xball[:, i, :], pab[:])
                for i in range(CHUNKS - STAG, CHUNKS):
                    tail(i)

    nc.compile()
    return nc


def _get_program():
    if "nc" not in _CACHE:
        _CACHE["nc"] = _build_program()
    return _CACHE["nc"]


def kernel(X, d, parameter_t, fm_mean, gamma, beta):
    from concourse.bass_utils import run_bass_kernel_spmd

    X = np.ascontiguousarray(np.asarray(X), dtype=np.float32)
    d = np.ascontiguousarray(np.asarray(d), dtype=np.int32)
    gamma = np.ascontiguousarray(np.asarray(gamma), dtype=np.float32)
    beta = np.ascontiguousarray(np.asarray(beta), dtype=np.float32)

    nc = _get_program()
    in_maps = [
        {
            "X": X[c * SHARD:(c + 1) * SHARD],
            "d": d[c * SHARD:(c + 1) * SHARD],
            "gamma": gamma,
            "beta": beta,
        }
        for c in range(NCORES)
    ]
    res = run_bass_kernel_spmd(nc, in_maps, core_ids=list(range(NCORES)))
    out = np.concatenate([res.results[c]["Y"] for c in range(NCORES)], axis=0)
    return out.astype(np.float32, copy=False)


# revision 7
# speedup vs baseline: 1.1202x; 1.1202x over previous
"""Per-domain batch normalization (BaseDomainBatchNorm) on 8 Trainium2 NeuronCores.

Math (reference):
    cnt[j]   = #{n : d[n] == j}            (clamped to >= 1)
    mean[j]  = sum_{d[n]==j} X[n] / cnt[j]
    var[j]   = sum_{d[n]==j} X[n]^2 / cnt[j] - mean[j]^2
    inv[j]   = rsqrt(var[j] + 1e-5)
    Y[n]     = (X[n] - mean[d[n]]) * inv[d[n]] * gamma[d[n]] + beta[d[n]]
             = X[n] * A[d[n]] + B[d[n]],  A = inv*gamma, B = beta - mean*A

Sharding: rows (samples) split 8192 per core; per-domain partial stats
(sum / sumsq / count) are AllReduce'd across the 8 cores; each core then
normalizes its own rows.  gamma/beta replicated.

Per-core pipeline (64 chunks of 128 rows; chunk i, partition p = row p*64+i):
  phase 1:  X arrives via SWDGE cast-DMAs directly as bf16 group tiles
            (HBM fp32 read at line rate, bf16 landed in SBUF); squares on
            DVE/ACT; one-hot stats matmuls (K=128) accumulate sum/sumsq.
  AR:       [16, 1025] partial stats AllReduce'd via collective_compute.
  finalize: A = gamma*rsqrt(var+eps) via ACT Abs_reciprocal_sqrt;
            B = beta - mean*A; tables bf16 in rows 0:16 of K=128
            zero-padded tiles (full-K matmuls keep the HAM clock-gate
            seeing real work; bf16 table entries gather EXACTLY).
  phase 2:  per chunk: PE gathers A-rows and B-rows (K=128 one-hot
            matmuls); ACT casts the A-gather PSUM->SBUF bf16 (lossless);
            DVE computes m = xb * a_rows (bf16 2x); PE adds m into the
            B-gather bank via an identity matmul; DVE evacuates Y bf16;
            SWDGE cast-DMAs store Y as fp32 (HBM write at line rate).
HBM traffic is the roofline minimum: read X once, write Y once.
"""

import numpy as np

N = 65536
C = 512
D = 16
NCORES = 8
SHARD = N // NCORES          # 8192 rows per core
P = 128                      # partitions
CHUNKS = SHARD // P          # 64 chunks of 128 rows
GRP = 4                      # chunks per X/Y DMA group (1 MiB fp32)
NGRP = CHUNKS // GRP         # 16 groups
EPS = 1e-5
STAG = 3                     # phase-2 software pipeline stagger (chunks)

_CACHE = {}


def _build_program():
    import concourse.bacc as bacc
    import concourse.bass as bass
    import concourse.tile as tile
    from concourse import mybir

    f32 = mybir.dt.float32
    bf16 = mybir.dt.bfloat16
    i32 = mybir.dt.int32
    Alu = mybir.AluOpType
    Act = mybir.ActivationFunctionType

    nc = bacc.Bacc("TRN2", target_bir_lowering=False, debug=False,
                   num_devices=NCORES)

    X_d = nc.dram_tensor("X", [SHARD, C], f32, kind="ExternalInput")
    d_d = nc.dram_tensor("d", [SHARD], i32, kind="ExternalInput")
    g_d = nc.dram_tensor("gamma", [D, C], f32, kind="ExternalInput")
    b_d = nc.dram_tensor("beta", [D, C], f32, kind="ExternalInput")
    Y_d = nc.dram_tensor("Y", [SHARD, C], f32, kind="ExternalOutput")

    cc_in = nc.dram_tensor("cc_in", [D, 2 * C + 1], f32)
    cc_out = nc.dram_tensor("cc_out", [D, 2 * C + 1], f32, addr_space="Shared")

    # partition p owns rows [p*64, (p+1)*64): per-partition contiguous DMA
    Xv = X_d.ap().rearrange("(p n) c -> p n c", p=P)   # [128, 64, 512]
    Yv = Y_d.ap().rearrange("(p n) c -> p n c", p=P)

    DB = 2048  # d-broadcast strip width for the transposed one-hot build

    with tile.TileContext(nc) as tc:
        with (
            tc.tile_pool(name="const", bufs=1) as cpool,
            tc.tile_pool(name="xb", bufs=1) as xbpool,
            tc.tile_pool(name="sq", bufs=3) as sqpool,
            tc.tile_pool(name="oh", bufs=1) as ohpool,
            tc.tile_pool(name="dbc", bufs=2) as dbcpool,
            tc.tile_pool(name="small", bufs=1) as spool,
            tc.tile_pool(name="scr", bufs=2) as scrpool,
            tc.tile_pool(name="pab", bufs=3) as pabpool,
            tc.tile_pool(name="m", bufs=STAG + 2) as mpool,
            tc.tile_pool(name="y", bufs=3) as ypool,
        ):
            # ---- X load: SWDGE cast-DMAs (fp32 HBM -> bf16 SBUF), first ----
            xbs = []
            for g in range(NGRP):
                xb = xbpool.tile([P, GRP, C], bf16, tag=f"xb{g}",
                                 name=f"xb{g}")
                xbs.append(xb)
                nc.gpsimd.dma_start(xb[:], Xv[:, GRP * g:GRP * (g + 1), :])

            # ---- constants ----
            # iota_rep[p, i, j] = j  (for the chunk-layout one-hot)
            iota_rep = cpool.tile([P, CHUNKS, D], bf16)
            nc.gpsimd.iota(iota_rep[:], pattern=[[0, CHUNKS], [1, D]], base=0,
                           channel_multiplier=0,
                           allow_small_or_imprecise_dtypes=True)
            # iota_col[p, 0] = p % 16 as f32 (for the transposed one-hot)
            iota_i = cpool.tile([P, 1], i32)
            nc.gpsimd.iota(iota_i[:], pattern=[[0, 1]], base=0,
                           channel_multiplier=1)
            nc.vector.tensor_scalar(iota_i[:], iota_i[:], D - 1, None,
                                    Alu.bitwise_and)
            iota_col = cpool.tile([P, 1], f32)
            nc.vector.tensor_copy(iota_col[:], iota_i[:])
            ones_col = cpool.tile([P, 1], bf16)
            nc.vector.memset(ones_col[:], 1.0)
            # identity (bf16) for the phase-2 add-via-matmul
            irow = cpool.tile([P, P], i32)
            nc.gpsimd.iota(irow[:], pattern=[[1, P]], base=0,
                           channel_multiplier=0)
            icol = cpool.tile([P, 1], i32)
            nc.gpsimd.iota(icol[:], pattern=[[0, 1]], base=0,
                           channel_multiplier=1)
            icolf = cpool.tile([P, 1], f32)
            nc.vector.tensor_copy(icolf[:], icol[:])
            ident = cpool.tile([P, P], bf16)
            nc.vector.tensor_scalar(ident[:], irow[:], icolf[:], None,
                                    Alu.is_equal)
            # K=128 zero-padded bf16 gather tables (rows 0:16 filled later)
            tab_a = cpool.tile([P, C], bf16)
            nc.vector.memset(tab_a[:], 0.0)
            tab_b = cpool.tile([P, C], bf16)
            nc.vector.memset(tab_b[:], 0.0)
            # feed tile for the keep-PE-warm matmuls (dep-gated below)
            wfeed = cpool.tile([P, C], bf16)
            nc.vector.memset(wfeed[:], 1.0)

            # ---- d in chunk layout ([p, n]) and one-hot [128, 64, 16] ----
            d_pn = cpool.tile([P, CHUNKS], i32)
            nc.sync.dma_start(d_pn[:], d_d.ap().rearrange("(p n) -> p n", p=P))
            d_f = cpool.tile([P, CHUNKS], bf16)
            nc.vector.tensor_copy(d_f[:], d_pn[:])
            onehot = ohpool.tile([P, CHUNKS, D], bf16)
            nc.vector.tensor_tensor(
                onehot[:], iota_rep[:],
                d_f[:].unsqueeze(-1).broadcast_to([P, CHUNKS, D]),
                Alu.is_equal)

            # ---- transposed one-hot, K=128 (row 32k+j holds onehot of
            # domain j=p%16; only rows 0:16 pair with nonzero table rows).
            # Column n = p*64 + i (natural shard order).
            ohT = ohpool.tile([P, SHARD], bf16)
            for h in range(SHARD // DB):
                d_bc = dbcpool.tile([P, DB], i32, tag="d_bc", name="d_bc")
                src = d_d.ap()[h * DB:(h + 1) * DB]
                src = src.rearrange("(a n) -> a n", a=1).partition_broadcast(P)
                nc.gpsimd.dma_start(d_bc[:], src)
                nc.vector.tensor_scalar(ohT[:, h * DB:(h + 1) * DB],
                                        d_bc[:], iota_col[:], None,
                                        Alu.is_equal)

            # counts: reduce one-hot over chunks (independent of X; early)
            rowcnt = spool.tile([P, D], f32, tag="rowcnt")
            nc.vector.tensor_reduce(
                rowcnt[:], onehot[:].rearrange("p n d -> p d n"),
                mybir.AxisListType.X, Alu.add)
            rowcnt_bf = spool.tile([P, D], bf16, tag="rowcnt_bf")
            nc.vector.tensor_copy(rowcnt_bf[:], rowcnt[:])

            # gamma/beta prefetch
            gam = spool.tile([D, C], f32, tag="gam")
            nc.sync.dma_start(gam[:], g_d[:])
            bet = spool.tile([D, C], f32, tag="bet")
            nc.sync.dma_start(bet[:], b_d[:])

            # ---- phase 1: per-core partial stats ----
            stats = spool.tile([D, 2 * C + 1], f32, tag="stats")
            with tc.tile_pool(name="ps1", bufs=1, space="PSUM") as ps1:
                psum_s = ps1.tile([D, C], f32)
                psum_q = ps1.tile([D, C], f32)
                psum_c = ps1.tile([D, 1], f32)
                for g in range(NGRP):
                    xbg = xbs[g][:].rearrange("p n c -> p (n c)")
                    xsq = sqpool.tile([P, GRP * C], bf16, tag="xsq")
                    if g % 2 == 0:
                        nc.scalar.activation(xsq[:], xbg, Act.Square)
                    else:
                        nc.vector.tensor_mul(xsq[:], xbg, xbg)
                    for k in range(GRP):
                        i = GRP * g + k
                        oh = onehot[:, i, :]
                        st, sp = (i == 0), (i == CHUNKS - 1)
                        nc.tensor.matmul(psum_s[:], oh, xbs[g][:, k, :],
                                         start=st, stop=sp)
                        nc.tensor.matmul(psum_q[:], oh,
                                         xsq[:, k * C:(k + 1) * C],
                                         start=st, stop=sp)

                nc.tensor.matmul(psum_c[:], rowcnt_bf[:], ones_col[:],
                                 start=True, stop=True)

                # ---- copy stats out of PSUM before freeing it ----
                nc.vector.tensor_copy(stats[:, 0:C], psum_s[:])
                nc.vector.tensor_copy(stats[:, C:2 * C], psum_q[:])
                nc.vector.tensor_copy(stats[:, 2 * C:2 * C + 1], psum_c[:])

                # keep the PE HAM clock-gate warm into the all-reduce stall.
                # wfeed's touch-up below depends on the stats copies (WAW on
                # the stats tile via subtile-coarse tracking is not needed:
                # the memset is ordered after the copies on the DVE stream),
                # which pins these matmuls after phase 1 in the PE stream.
                nc.vector.memset(wfeed[:, 0:1], 1.0)
                warm = ps1.tile([P, C], f32)
                for _ in range(10):
                    nc.tensor.matmul(warm[:], ident[:], wfeed[:],
                                     start=True, stop=True,
                                     skip_group_check=True)

            # ---- all-reduce partial stats across the 8 cores ----
            nc.sync.dma_start(cc_in[:], stats[:])
            nc.gpsimd.collective_compute(
                "AllReduce", Alu.add,
                replica_groups=[list(range(NCORES))],
                ins=[cc_in[:]], outs=[cc_out[:]])
            red = spool.tile([D, 2 * C + 1], f32, tag="red")
            nc.sync.dma_start(red[:], cc_out[:])

            # ---- finalize: A = inv*gamma, B = beta - mean*A ----
            cntc = spool.tile([D, 1], f32, tag="cntc")
            nc.vector.tensor_scalar_max(cntc[:], red[:, 2 * C:2 * C + 1], 1.0)
            rinv = spool.tile([D, 1], f32, tag="rinv")
            nc.vector.reciprocal(rinv[:], cntc[:])
            # mean | E[x^2] in one strip: [16, 1024]
            mm = spool.tile([D, 2 * C], f32, tag="mm")
            nc.vector.tensor_scalar_mul(mm[:], red[:, 0:2 * C], rinv[:])
            mean = mm[:, 0:C]
            var = spool.tile([D, C], f32, tag="var")
            nc.vector.scalar_tensor_tensor(var[:], mean, -1.0, mean,
                                           Alu.mult, Alu.mult)   # -mean^2
            nc.vector.tensor_add(var[:], var[:], mm[:, C:2 * C])
            epsb = spool.tile([D, 1], f32, tag="epsb")
            nc.vector.memset(epsb[:], EPS)
            # inv = rsqrt(var + eps)  (var+eps > 0, so |.| is a no-op)
            inv = spool.tile([D, C], f32, tag="inv")
            nc.scalar.activation(inv[:], var[:], Act.Abs_reciprocal_sqrt,
                                 bias=epsb[:])
            a_t = spool.tile([D, C], f32, tag="a_t")
            nc.vector.tensor_mul(a_t[:], inv[:], gam[:])
            negma = scrpool.tile([D, C], f32, tag="scr")
            nc.vector.scalar_tensor_tensor(negma[:], mean, -1.0, a_t[:],
                                           Alu.mult, Alu.mult)   # -mean*A
            b_t = spool.tile([D, C], f32, tag="b_t")
            nc.vector.tensor_add(b_t[:], bet[:], negma[:])

            # bf16 tables into rows 0:16 of the zero-padded K=128 tiles
            nc.vector.tensor_copy(tab_a[0:D, :], a_t[:])
            nc.scalar.activation(tab_b[0:D, :], b_t[:], Act.Copy)

            ohTv = ohT[:].rearrange("k (p i) -> k i p", i=CHUNKS)

            # ---- phase 2: gather A/B per row, Y = X*A + B ----
            with tc.tile_pool(name="ps2a", bufs=3, space="PSUM") as ps2a, \
                 tc.tile_pool(name="ps2b", bufs=3, space="PSUM") as ps2b:
                psa = [None] * CHUNKS
                psb = [None] * CHUNKS
                mt = [None] * CHUNKS
                yg = None

                def tail(i):
                    # add m into the B-gather bank, then evacuate Y (bf16)
                    nonlocal yg
                    nc.tensor.matmul(psb[i][:], ident[:], mt[i][:],
                                     start=False, stop=True)
                    g, k = divmod(i, GRP)
                    if k == 0:
                        yg = ypool.tile([P, GRP, C], bf16, tag="yg",
                                        name="yg")
                    nc.vector.tensor_copy(yg[:, k, :], psb[i][:])
                    if k == GRP - 1:
                        nc.gpsimd.dma_start(Yv[:, GRP * g:GRP * (g + 1), :],
                                            yg[:])

                for i in range(CHUNKS):
                    g, k = divmod(i, GRP)
                    psa[i] = ps2a.tile([P, C], f32, tag="psa", name="psa")
                    psb[i] = ps2b.tile([P, C], f32, tag="psb", name="psb")
                    nc.tensor.matmul(psa[i][:], ohTv[:, i, :], tab_a[:],
                                     start=True, stop=True)
                    nc.tensor.matmul(psb[i][:], ohTv[:, i, :], tab_b[:],
                                     start=True, stop=False)
                    if i >= STAG:
                        tail(i - STAG)
                    pab = pabpool.tile([P, C], bf16, tag="pab", name="pab")
                    nc.scalar.activation(pab[:], psa[i][:], Act.Copy)
                    mt[i] = mpool.tile([P, C], bf16, tag="mt", name="mt")
                    nc.vector.tensor_mul(mt[i][:], xbs[g][:, k, :], pab[:])
                for i in range(CHUNKS - STAG, CHUNKS):
                    tail(i)

    nc.compile()
    return nc


def _get_program():
    if "nc" not in _CACHE:
        _CACHE["nc"] = _build_program()
    return _CACHE["nc"]


def kernel(X, d, parameter_t, fm_mean, gamma, beta):
    from concourse.bass_utils import run_bass_kernel_spmd

    X = np.ascontiguousarray(np.asarray(X), dtype=np.float32)
    d = np.ascontiguousarray(np.asarray(d), dtype=np.int32)
    gamma = np.ascontiguousarray(np.asarray(gamma), dtype=np.float32)
    beta = np.ascontiguousarray(np.asarray(beta), dtype=np.float32)

    nc = _get_program()
    in_maps = [
        {
            "X": X[c * SHARD:(c + 1) * SHARD],
            "d": d[c * SHARD:(c + 1) * SHARD],
            "gamma": gamma,
            "beta": beta,
        }
        for c in range(NCORES)
    ]
    res = run_bass_kernel_spmd(nc, in_maps, core_ids=list(range(NCORES)))
    out = np.concatenate([res.results[c]["Y"] for c in range(NCORES)], axis=0)
    return out.astype(np.float32, copy=False)


# revision 8
# speedup vs baseline: 1.2276x; 1.0959x over previous
"""Per-domain batch normalization (BaseDomainBatchNorm) on 8 Trainium2 NeuronCores.

Math (reference):
    cnt[j]   = #{n : d[n] == j}            (clamped to >= 1)
    mean[j]  = sum_{d[n]==j} X[n] / cnt[j]
    var[j]   = sum_{d[n]==j} X[n]^2 / cnt[j] - mean[j]^2
    inv[j]   = rsqrt(var[j] + 1e-5)
    Y[n]     = (X[n] - mean[d[n]]) * inv[d[n]] * gamma[d[n]] + beta[d[n]]
             = X[n] * A[d[n]] + B[d[n]],  A = inv*gamma, B = beta - mean*A

Sharding: rows (samples) split 8192 per core; per-domain partial stats
(sum / sumsq / count) are AllReduce'd across the 8 cores; each core then
normalizes its own rows.  gamma/beta replicated.

Per-core pipeline (64 chunks of 128 rows; chunk i, partition p = row p*64+i):
  phase 1:  X arrives via HWDGE fp32 group DMAs; DVE casts to resident
            bf16 group tiles; squares on DVE/ACT; one-hot stats matmuls
            (K=128) accumulate sum/sumsq.
  AR:       [16, 1025] partial stats AllReduce'd via collective_compute.
  finalize: A = gamma*rsqrt(var+eps) via ACT Abs_reciprocal_sqrt;
            B = beta - mean*A; tables bf16 in rows 0:16 of K=128
            zero-padded tiles (full-K matmuls keep the HAM clock-gate
            seeing real work; bf16 table entries gather EXACTLY).
  phase 2:  per chunk: PE gathers A-rows and B-rows (K=128 one-hot
            matmuls); DVE computes m = xb * a_rows reading the A-gather
            PSUM directly; PE adds m into the B-gather bank via an
            identity matmul; ACT evacuates Y fp32; HWDGE stores Y.
HBM traffic is the roofline minimum: read X once, write Y once.
"""

import numpy as np

N = 65536
C = 512
D = 16
NCORES = 8
SHARD = N // NCORES          # 8192 rows per core
P = 128                      # partitions
CHUNKS = SHARD // P          # 64 chunks of 128 rows
GRP = 4                      # chunks per X/Y DMA group (1 MiB fp32)
NGRP = CHUNKS // GRP         # 16 groups
EPS = 1e-5
STAG = 3                     # phase-2 software pipeline stagger (chunks)

_CACHE = {}


def _build_program():
    import concourse.bacc as bacc
    import concourse.bass as bass
    import concourse.tile as tile
    from concourse import mybir

    f32 = mybir.dt.float32
    bf16 = mybir.dt.bfloat16
    i32 = mybir.dt.int32
    Alu = mybir.AluOpType
    Act = mybir.ActivationFunctionType

    nc = bacc.Bacc("TRN2", target_bir_lowering=False, debug=False,
                   num_devices=NCORES)

    X_d = nc.dram_tensor("X", [SHARD, C], f32, kind="ExternalInput")
    d_d = nc.dram_tensor("d", [SHARD], i32, kind="ExternalInput")
    g_d = nc.dram_tensor("gamma", [D, C], f32, kind="ExternalInput")
    b_d = nc.dram_tensor("beta", [D, C], f32, kind="ExternalInput")
    Y_d = nc.dram_tensor("Y", [SHARD, C], f32, kind="ExternalOutput")

    cc_in = nc.dram_tensor("cc_in", [D, 2 * C + 1], f32)
    cc_out = nc.dram_tensor("cc_out", [D, 2 * C + 1], f32, addr_space="Shared")

    # partition p owns rows [p*64, (p+1)*64): per-partition contiguous DMA
    Xv = X_d.ap().rearrange("(p n) c -> p n c", p=P)   # [128, 64, 512]
    Yv = Y_d.ap().rearrange("(p n) c -> p n c", p=P)

    DB = 2048  # d-broadcast strip width for the transposed one-hot build

    with tile.TileContext(nc) as tc:
        with (
            tc.tile_pool(name="const", bufs=1) as cpool,
            tc.tile_pool(name="xg", bufs=3) as xgpool,
            tc.tile_pool(name="xb", bufs=1) as xbpool,
            tc.tile_pool(name="sq", bufs=3) as sqpool,
            tc.tile_pool(name="oh", bufs=1) as ohpool,
            tc.tile_pool(name="dbc", bufs=2) as dbcpool,
            tc.tile_pool(name="small", bufs=1) as spool,
            tc.tile_pool(name="scr", bufs=2) as scrpool,
            tc.tile_pool(name="pab", bufs=3) as pabpool,
            tc.tile_pool(name="m", bufs=STAG + 2) as mpool,
            tc.tile_pool(name="y", bufs=3) as ypool,
        ):
            # ---- X load: HWDGE fp32 group DMAs, issued first ----
            xgs = []
            xbs = []
            for g in range(NGRP):
                xg = xgpool.tile([P, GRP, C], f32, tag="xg", name="xg")
                xgs.append(xg)
                nc.sync.dma_start(xg[:], Xv[:, GRP * g:GRP * (g + 1), :])
                xb = xbpool.tile([P, GRP, C], bf16, tag=f"xb{g}",
                                 name=f"xb{g}")
                xbs.append(xb)

            # ---- constants ----
            # iota_rep[p, i, j] = j  (for the chunk-layout one-hot)
            iota_rep = cpool.tile([P, CHUNKS, D], bf16)
            nc.gpsimd.iota(iota_rep[:], pattern=[[0, CHUNKS], [1, D]], base=0,
                           channel_multiplier=0,
                           allow_small_or_imprecise_dtypes=True)
            # iota_col[p, 0] = p % 16 as f32 (for the transposed one-hot)
            iota_i = cpool.tile([P, 1], i32)
            nc.gpsimd.iota(iota_i[:], pattern=[[0, 1]], base=0,
                           channel_multiplier=1)
            nc.vector.tensor_scalar(iota_i[:], iota_i[:], D - 1, None,
                                    Alu.bitwise_and)
            iota_col = cpool.tile([P, 1], f32)
            nc.vector.tensor_copy(iota_col[:], iota_i[:])
            ones_col = cpool.tile([P, 1], bf16)
            nc.vector.memset(ones_col[:], 1.0)
            # identity (bf16) for the phase-2 add-via-matmul
            irow = cpool.tile([P, P], i32)
            nc.gpsimd.iota(irow[:], pattern=[[1, P]], base=0,
                           channel_multiplier=0)
            icol = cpool.tile([P, 1], i32)
            nc.gpsimd.iota(icol[:], pattern=[[0, 1]], base=0,
                           channel_multiplier=1)
            icolf = cpool.tile([P, 1], f32)
            nc.vector.tensor_copy(icolf[:], icol[:])
            ident = cpool.tile([P, P], bf16)
            nc.vector.tensor_scalar(ident[:], irow[:], icolf[:], None,
                                    Alu.is_equal)
            # K=128 zero-padded bf16 gather tables (rows 0:16 filled later)
            tab_a = cpool.tile([P, C], bf16)
            nc.vector.memset(tab_a[:], 0.0)
            tab_b = cpool.tile([P, C], bf16)
            nc.vector.memset(tab_b[:], 0.0)
            # feed tile for the keep-PE-warm matmuls (dep-gated below)
            wfeed = cpool.tile([P, C], bf16)
            nc.vector.memset(wfeed[:], 1.0)

            # ---- d in chunk layout ([p, n]) and one-hot [128, 64, 16] ----
            d_pn = cpool.tile([P, CHUNKS], i32)
            nc.scalar.dma_start(d_pn[:], d_d.ap().rearrange("(p n) -> p n", p=P))
            d_f = cpool.tile([P, CHUNKS], bf16)
            nc.vector.tensor_copy(d_f[:], d_pn[:])
            onehot = ohpool.tile([P, CHUNKS, D], bf16)
            nc.vector.tensor_tensor(
                onehot[:], iota_rep[:],
                d_f[:].unsqueeze(-1).broadcast_to([P, CHUNKS, D]),
                Alu.is_equal)

            # ---- transposed one-hot, K=128 (row 32k+j holds onehot of
            # domain j=p%16; only rows 0:16 pair with nonzero table rows).
            # Column n = p*64 + i (natural shard order).
            ohT = ohpool.tile([P, SHARD], bf16)
            for h in range(SHARD // DB):
                d_bc = dbcpool.tile([P, DB], i32, tag="d_bc", name="d_bc")
                src = d_d.ap()[h * DB:(h + 1) * DB]
                src = src.rearrange("(a n) -> a n", a=1).partition_broadcast(P)
                nc.gpsimd.dma_start(d_bc[:], src)
                nc.vector.tensor_scalar(ohT[:, h * DB:(h + 1) * DB],
                                        d_bc[:], iota_col[:], None,
                                        Alu.is_equal)

            # counts: reduce one-hot over chunks (independent of X; early)
            rowcnt = spool.tile([P, D], f32, tag="rowcnt")
            nc.vector.tensor_reduce(
                rowcnt[:], onehot[:].rearrange("p n d -> p d n"),
                mybir.AxisListType.X, Alu.add)
            rowcnt_bf = spool.tile([P, D], bf16, tag="rowcnt_bf")
            nc.vector.tensor_copy(rowcnt_bf[:], rowcnt[:])

            # gamma/beta prefetch
            gam = spool.tile([D, C], f32, tag="gam")
            nc.scalar.dma_start(gam[:], g_d[:])
            bet = spool.tile([D, C], f32, tag="bet")
            nc.scalar.dma_start(bet[:], b_d[:])

            # ---- phase 1: per-core partial stats ----
            stats = spool.tile([D, 2 * C + 1], f32, tag="stats")
            with tc.tile_pool(name="ps1", bufs=1, space="PSUM") as ps1:
                psum_s = ps1.tile([D, C], f32)
                psum_q = ps1.tile([D, C], f32)
                psum_c = ps1.tile([D, 1], f32)
                for g in range(NGRP):
                    xbg = xbs[g][:].rearrange("p n c -> p (n c)")
                    nc.vector.tensor_copy(
                        xbg, xgs[g][:].rearrange("p n c -> p (n c)"))
                    xsq = sqpool.tile([P, GRP * C], bf16, tag="xsq")
                    if g % 2 == 0:
                        nc.scalar.activation(xsq[:], xbg, Act.Square)
                    else:
                        nc.vector.tensor_mul(xsq[:], xbg, xbg)
                    for k in range(GRP):
                        i = GRP * g + k
                        oh = onehot[:, i, :]
                        st, sp = (i == 0), (i == CHUNKS - 1)
                        nc.tensor.matmul(psum_s[:], oh, xbs[g][:, k, :],
                                         start=st, stop=sp)
                        nc.tensor.matmul(psum_q[:], oh,
                                         xsq[:, k * C:(k + 1) * C],
                                         start=st, stop=sp)

                nc.tensor.matmul(psum_c[:], rowcnt_bf[:], ones_col[:],
                                 start=True, stop=True)

                # ---- copy stats out of PSUM before freeing it ----
                nc.vector.tensor_copy(stats[:, 0:C], psum_s[:])
                nc.vector.tensor_copy(stats[:, C:2 * C], psum_q[:])
                nc.vector.tensor_copy(stats[:, 2 * C:2 * C + 1], psum_c[:])

                # keep the PE HAM clock-gate warm into the all-reduce stall.
                # wfeed's touch-up below depends on the stats copies (WAW on
                # the stats tile via subtile-coarse tracking is not needed:
                # the memset is ordered after the copies on the DVE stream),
                # which pins these matmuls after phase 1 in the PE stream.
                nc.vector.memset(wfeed[:, 0:1], 1.0)
                warm = ps1.tile([P, C], f32)
                for _ in range(10):
                    nc.tensor.matmul(warm[:], ident[:], wfeed[:],
                                     start=True, stop=True,
                                     skip_group_check=True)

            # ---- all-reduce partial stats across the 8 cores ----
            nc.scalar.dma_start(cc_in[:], stats[:])
            nc.gpsimd.collective_compute(
                "AllReduce", Alu.add,
                replica_groups=[list(range(NCORES))],
                ins=[cc_in[:]], outs=[cc_out[:]])
            red = spool.tile([D, 2 * C + 1], f32, tag="red")
            nc.scalar.dma_start(red[:], cc_out[:])

            # ---- finalize: A = inv*gamma, B = beta - mean*A ----
            cntc = spool.tile([D, 1], f32, tag="cntc")
            nc.vector.tensor_scalar_max(cntc[:], red[:, 2 * C:2 * C + 1], 1.0)
            rinv = spool.tile([D, 1], f32, tag="rinv")
            nc.vector.reciprocal(rinv[:], cntc[:])
            # mean | E[x^2] in one strip: [16, 1024]
            mm = spool.tile([D, 2 * C], f32, tag="mm")
            nc.vector.tensor_scalar_mul(mm[:], red[:, 0:2 * C], rinv[:])
            mean = mm[:, 0:C]
            var = spool.tile([D, C], f32, tag="var")
            nc.vector.scalar_tensor_tensor(var[:], mean, -1.0, mean,
                                           Alu.mult, Alu.mult)   # -mean^2
            nc.vector.tensor_add(var[:], var[:], mm[:, C:2 * C])
            epsb = spool.tile([D, 1], f32, tag="epsb")
            nc.vector.memset(epsb[:], EPS)
            # inv = rsqrt(var + eps)  (var+eps > 0, so |.| is a no-op)
            inv = spool.tile([D, C], f32, tag="inv")
            nc.scalar.activation(inv[:], var[:], Act.Abs_reciprocal_sqrt,
                                 bias=epsb[:])
            a_t = spool.tile([D, C], f32, tag="a_t")
            nc.vector.tensor_mul(a_t[:], inv[:], gam[:])
            negma = scrpool.tile([D, C], f32, tag="scr")
            nc.vector.scalar_tensor_tensor(negma[:], mean, -1.0, a_t[:],
                                           Alu.mult, Alu.mult)   # -mean*A
            b_t = spool.tile([D, C], f32, tag="b_t")
            nc.vector.tensor_add(b_t[:], bet[:], negma[:])

            # bf16 tables into rows 0:16 of the zero-padded K=128 tiles
            nc.vector.tensor_copy(tab_a[0:D, :], a_t[:])
            nc.scalar.activation(tab_b[0:D, :], b_t[:], Act.Copy)

            ohTv = ohT[:].rearrange("k (p i) -> k i p", i=CHUNKS)

            # ---- phase 2: gather A/B per row, Y = X*A + B ----
            with tc.tile_pool(name="ps2a", bufs=3, space="PSUM") as ps2a, \
                 tc.tile_pool(name="ps2b", bufs=3, space="PSUM") as ps2b:
                psa = [None] * CHUNKS
                psb = [None] * CHUNKS
                mt = [None] * CHUNKS
                yg = None

                def tail(i):
                    # add m into the B-gather bank, then evacuate Y (fp32)
                    nonlocal yg
                    nc.tensor.matmul(psb[i][:], ident[:], mt[i][:],
                                     start=False, stop=True)
                    g, k = divmod(i, GRP)
                    if k == 0:
                        yg = ypool.tile([P, GRP, C], f32, tag="yg",
                                        name="yg")
                    nc.scalar.activation(yg[:, k, :], psb[i][:], Act.Copy)
                    if k == GRP - 1:
                        nc.sync.dma_start(Yv[:, GRP * g:GRP * (g + 1), :],
                                          yg[:])

                for i in range(CHUNKS):
                    g, k = divmod(i, GRP)
                    psa[i] = ps2a.tile([P, C], f32, tag="psa", name="psa")
                    psb[i] = ps2b.tile([P, C], f32, tag="psb", name="psb")
                    nc.tensor.matmul(psa[i][:], ohTv[:, i, :], tab_a[:],
                                     start=True, stop=True)
                    nc.tensor.matmul(psb[i][:], ohTv[:, i, :], tab_b[:],
                                     start=True, stop=False)
                    if i >= STAG:
                        tail(i - STAG)
                    mt[i] = mpool.tile([P, C], bf16, tag="mt", name="mt")
                    nc.vector.tensor_mul(mt[i][:], xbs[g][:, k, :],
                                         psa[i][:])
                for i in range(CHUNKS - STAG, CHUNKS):
                    tail(i)

    nc.compile()
    return nc


def _get_program():
    if "nc" not in _CACHE:
        _CACHE["nc"] = _build_program()
    return _CACHE["nc"]


def kernel(X, d, parameter_t, fm_mean, gamma, beta):
    from concourse.bass_utils import run_bass_kernel_spmd

    X = np.ascontiguousarray(np.asarray(X), dtype=np.float32)
    d = np.ascontiguousarray(np.asarray(d), dtype=np.int32)
    gamma = np.ascontiguousarray(np.asarray(gamma), dtype=np.float32)
    beta = np.ascontiguousarray(np.asarray(beta), dtype=np.float32)

    nc = _get_program()
    in_maps = [
        {
            "X": X[c * SHARD:(c + 1) * SHARD],
            "d": d[c * SHARD:(c + 1) * SHARD],
            "gamma": gamma,
            "beta": beta,
        }
        for c in range(NCORES)
    ]
    res = run_bass_kernel_spmd(nc, in_maps, core_ids=list(range(NCORES)))
    out = np.concatenate([res.results[c]["Y"] for c in range(NCORES)], axis=0)
    return out.astype(np.float32, copy=False)


# revision 12
# speedup vs baseline: 1.2671x; 1.0321x over previous
"""Per-domain batch normalization (BaseDomainBatchNorm) on 8 Trainium2 NeuronCores.

Math (reference):
    cnt[j]   = #{n : d[n] == j}            (clamped to >= 1)
    mean[j]  = sum_{d[n]==j} X[n] / cnt[j]
    var[j]   = sum_{d[n]==j} X[n]^2 / cnt[j] - mean[j]^2
    inv[j]   = rsqrt(var[j] + 1e-5)
    Y[n]     = (X[n] - mean[d[n]]) * inv[d[n]] * gamma[d[n]] + beta[d[n]]
             = X[n] * A[d[n]] + B[d[n]],  A = inv*gamma, B = beta - mean*A

Sharding: rows (samples) split 8192 per core; per-domain partial stats
(sum / sumsq / count) are AllReduce'd across the 8 cores; each core then
normalizes its own rows.  gamma/beta replicated.

Per-core pipeline (64 chunks of 128 rows; chunk i, partition p = row p*64+i):
  phase 1:  X arrives via HWDGE fp32 group DMAs; DVE casts to resident
            bf16 group tiles; squares on DVE/ACT; one-hot stats matmuls
            (K=128) accumulate sum/sumsq.
  AR:       [16, 1025] partial stats AllReduce'd via collective_compute.
  finalize: A = gamma*rsqrt(var+eps) via ACT Abs_reciprocal_sqrt;
            B = beta - mean*A; tables bf16 in rows 0:16 of K=128
            zero-padded tiles (full-K matmuls keep the HAM clock-gate
            seeing real work; bf16 table entries gather EXACTLY).
  phase 2:  per chunk: PE gathers A-rows and B-rows (K=128 one-hot
            matmuls); DVE computes m = xb * a_rows reading the A-gather
            PSUM directly; PE adds m into the B-gather bank via an
            identity matmul; ACT evacuates Y fp32; HWDGE stores Y.
HBM traffic is the roofline minimum: read X once, write Y once.
"""

import numpy as np

N = 65536
C = 512
D = 16
NCORES = 8
SHARD = N // NCORES          # 8192 rows per core
P = 128                      # partitions
CHUNKS = SHARD // P          # 64 chunks of 128 rows
GRP = 4                      # chunks per X/Y DMA group (1 MiB fp32)
NGRP = CHUNKS // GRP         # 16 groups
EPS = 1e-5
STAG = 3                     # phase-2 software pipeline stagger (chunks)

_CACHE = {}


def _build_program():
    import concourse.bacc as bacc
    import concourse.bass as bass
    import concourse.tile as tile
    from concourse import mybir

    f32 = mybir.dt.float32
    bf16 = mybir.dt.bfloat16
    i32 = mybir.dt.int32
    Alu = mybir.AluOpType
    Act = mybir.ActivationFunctionType

    nc = bacc.Bacc("TRN2", target_bir_lowering=False, debug=False,
                   num_devices=NCORES)

    X_d = nc.dram_tensor("X", [SHARD, C], f32, kind="ExternalInput")
    d_d = nc.dram_tensor("d", [SHARD], i32, kind="ExternalInput")
    g_d = nc.dram_tensor("gamma", [D, C], f32, kind="ExternalInput")
    b_d = nc.dram_tensor("beta", [D, C], f32, kind="ExternalInput")
    Y_d = nc.dram_tensor("Y", [SHARD, C], f32, kind="ExternalOutput")

    cc_in = nc.dram_tensor("cc_in", [D, 2 * 8 * 65], f32)
    cc_out = nc.dram_tensor("cc_out", [D, 2 * 8 * 65], f32,
                            addr_space="Shared")

    # partition p owns rows [p*64, (p+1)*64): per-partition contiguous DMA
    Xv = X_d.ap().rearrange("(p n) c -> p n c", p=P)   # [128, 64, 512]
    Yv = Y_d.ap().rearrange("(p n) c -> p n c", p=P)

    DB = 2048  # d-broadcast strip width for the transposed one-hot build

    with tile.TileContext(nc) as tc:
        with (
            tc.tile_pool(name="const", bufs=1) as cpool,
            tc.tile_pool(name="xg", bufs=4) as xgpool,
            tc.tile_pool(name="xb", bufs=1) as xbpool,
            tc.tile_pool(name="sq", bufs=2) as sqpool,
            tc.tile_pool(name="oh", bufs=1) as ohpool,
            tc.tile_pool(name="dbc", bufs=2) as dbcpool,
            tc.tile_pool(name="small", bufs=1) as spool,
            tc.tile_pool(name="scr", bufs=2) as scrpool,
            tc.tile_pool(name="m", bufs=STAG + 2) as mpool,
            tc.tile_pool(name="y", bufs=3) as ypool,
        ):
            # ---- X load: HWDGE fp32 group DMAs, issued first ----
            xgs = []
            xbs = []
            for g in range(NGRP):
                xg = xgpool.tile([P, GRP, C], f32, tag="xg", name="xg")
                xgs.append(xg)
                nc.sync.dma_start(xg[:], Xv[:, GRP * g:GRP * (g + 1), :])
                xb = xbpool.tile([P, GRP, C], bf16, tag=f"xb{g}",
                                 name=f"xb{g}")
                xbs.append(xb)

            # ---- constants ----
            # iota_rep[p, i, j] = j  (for the chunk-layout one-hot)
            iota_rep = cpool.tile([P, CHUNKS, D], bf16)
            nc.gpsimd.iota(iota_rep[:], pattern=[[0, CHUNKS], [1, D]], base=0,
                           channel_multiplier=0,
                           allow_small_or_imprecise_dtypes=True)
            # iota_col[p, 0] = p % 16 as f32 (for the transposed one-hot)
            iota_i = cpool.tile([P, 1], i32)
            nc.gpsimd.iota(iota_i[:], pattern=[[0, 1]], base=0,
                           channel_multiplier=1)
            nc.vector.tensor_scalar(iota_i[:], iota_i[:], D - 1, None,
                                    Alu.bitwise_and)
            iota_col = cpool.tile([P, 1], f32)
            nc.vector.tensor_copy(iota_col[:], iota_i[:])
            ones_col = cpool.tile([P, 1], bf16)
            nc.vector.memset(ones_col[:], 1.0)
            # identity (bf16) for the phase-2 add-via-matmul
            irow = cpool.tile([P, P], i32)
            nc.gpsimd.iota(irow[:], pattern=[[1, P]], base=0,
                           channel_multiplier=0)
            icol = cpool.tile([P, 1], i32)
            nc.gpsimd.iota(icol[:], pattern=[[0, 1]], base=0,
                           channel_multiplier=1)
            icolf = cpool.tile([P, 1], f32)
            nc.vector.tensor_copy(icolf[:], icol[:])
            ident = cpool.tile([P, P], bf16)
            nc.vector.tensor_scalar(ident[:], irow[:], icolf[:], None,
                                    Alu.is_equal)
            # K=128 zero-padded bf16 gather tables (rows 0:16 filled later)
            tab_a = cpool.tile([P, C], bf16)
            nc.vector.memset(tab_a[:], 0.0)
            tab_b = cpool.tile([P, C], bf16)
            nc.vector.memset(tab_b[:], 0.0)
            # feed tile for the keep-PE-warm matmuls (dep-gated below)
            wfeed = cpool.tile([P, C], bf16)
            nc.vector.memset(wfeed[:], 1.0)

            # ---- d in chunk layout ([p, n]) and one-hot [128, 64, 16] ----
            d_pn = cpool.tile([P, CHUNKS], i32)
            nc.scalar.dma_start(d_pn[:], d_d.ap().rearrange("(p n) -> p n", p=P))
            d_f = cpool.tile([P, CHUNKS], bf16)
            nc.vector.tensor_copy(d_f[:], d_pn[:])
            onehot = ohpool.tile([P, CHUNKS, D], bf16)
            nc.vector.tensor_tensor(
                onehot[:], iota_rep[:],
                d_f[:].unsqueeze(-1).broadcast_to([P, CHUNKS, D]),
                Alu.is_equal)

            # ---- transposed one-hot, K=128 (row 32k+j holds onehot of
            # domain j=p%16; only rows 0:16 pair with nonzero table rows).
            # Column n = p*64 + i (natural shard order).
            ohT = ohpool.tile([P, SHARD], bf16)
            for h in range(SHARD // DB):
                d_bc = dbcpool.tile([P, DB], i32, tag="d_bc", name="d_bc")
                src = d_d.ap()[h * DB:(h + 1) * DB]
                src = src.rearrange("(a n) -> a n", a=1).partition_broadcast(P)
                nc.gpsimd.dma_start(d_bc[:], src)
                nc.vector.tensor_scalar(ohT[:, h * DB:(h + 1) * DB],
                                        d_bc[:], iota_col[:], None,
                                        Alu.is_equal)

            # counts: reduce one-hot over chunks (independent of X; early)
            rowcnt = spool.tile([P, D], f32, tag="rowcnt")
            nc.vector.tensor_reduce(
                rowcnt[:], onehot[:].rearrange("p n d -> p d n"),
                mybir.AxisListType.X, Alu.add)
            rowcnt_bf = spool.tile([P, D], bf16, tag="rowcnt_bf")
            nc.vector.tensor_copy(rowcnt_bf[:], rowcnt[:])

            # gamma/beta prefetch
            gam = spool.tile([D, C], f32, tag="gam")
            nc.scalar.dma_start(gam[:], g_d[:])
            bet = spool.tile([D, C], f32, tag="bet")
            nc.scalar.dma_start(bet[:], b_d[:])

            # ---- phase 1: per-core partial stats ----
            statsw = spool.tile([D, 2 * 8 * 65], f32, tag="statsw")
            redw = spool.tile([D, 2 * 8 * 65], f32, tag="redw")
            with tc.tile_pool(name="ps1", bufs=1, space="PSUM") as ps1:
                psum_s = ps1.tile([D, C], f32)
                psum_q = ps1.tile([D, C], f32)
                psum_c = ps1.tile([D, 1], f32)
                for g in range(NGRP):
                    xbg = xbs[g][:].rearrange("p n c -> p (n c)")
                    nc.vector.tensor_copy(
                        xbg, xgs[g][:].rearrange("p n c -> p (n c)"))
                    xsq = sqpool.tile([P, GRP * C], bf16, tag="xsq")
                    if g % 2 == 0:
                        nc.scalar.activation(xsq[:], xbg, Act.Square)
                    else:
                        nc.vector.tensor_mul(xsq[:], xbg, xbg)
                    for k in range(GRP):
                        i = GRP * g + k
                        oh = onehot[:, i, :]
                        st, sp = (i == 0), (i == CHUNKS - 1)
                        nc.tensor.matmul(psum_s[:], oh, xbs[g][:, k, :],
                                         start=st, stop=sp)
                        nc.tensor.matmul(psum_q[:], oh,
                                         xsq[:, k * C:(k + 1) * C],
                                         start=st, stop=sp)

                nc.tensor.matmul(psum_c[:], rowcnt_bf[:], ones_col[:],
                                 start=True, stop=True)

                # ---- copy stats out of PSUM in (d,b)-slot layout:
                # statsw[d, (g,b)*65 + f] = sum/sq of block b (f<64), cnt (f=64)
                svw = statsw[:].rearrange("d (g b f) -> d g b f", g=2, f=65)
                nc.vector.tensor_copy(
                    svw[:, 0, :, 0:64],
                    psum_s[:].rearrange("d (b f) -> d b f", f=64))
                nc.vector.tensor_copy(
                    svw[:, 1, :, 0:64],
                    psum_q[:].rearrange("d (b f) -> d b f", f=64))
                nc.vector.tensor_copy(
                    statsw[:].rearrange("d (g f) -> d g f", f=65)[:, :, 64:65],
                    psum_c[:].unsqueeze(1).broadcast_to([D, 2 * 8, 1]))

                # keep the PE HAM clock-gate warm into the all-reduce stall.
                # wfeed's touch-up below depends on the stats copies (WAW on
                # the stats tile via subtile-coarse tracking is not needed:
                # the memset is ordered after the copies on the DVE stream),
                # which pins these matmuls after phase 1 in the PE stream.
                nc.vector.memset(wfeed[:, 0:1], 1.0)
                warm = ps1.tile([P, C], f32)
                for _ in range(10):
                    nc.tensor.matmul(warm[:], ident[:], wfeed[:],
                                     start=True, stop=True,
                                     skip_group_check=True)

            # ---- all-reduce partial stats across the 8 cores ----
            nc.scalar.dma_start(cc_in[:], statsw[:])
            nc.gpsimd.collective_compute(
                "AllReduce", Alu.add,
                replica_groups=[list(range(NCORES))],
                ins=[cc_in[:]], outs=[cc_out[:]])
            nc.scalar.dma_start(redw[:], cc_out.ap())

            # ---- finalize: A = inv*gamma, B = beta - mean*A ----
            cntc = spool.tile([D, 1], f32, tag="cntc")
            nc.vector.tensor_scalar_max(cntc[:], redw[:, 64:65], 1.0)
            rinv = spool.tile([D, 1], f32, tag="rinv")
            nc.vector.reciprocal(rinv[:], cntc[:])
            # mean | E[x^2] in one strip: [16, 1024] (c = b*64 + f)
            mm = spool.tile([D, 2 * C], f32, tag="mm")
            nc.vector.tensor_scalar_mul(
                mm[:],
                redw[:].rearrange("d (g b f) -> d g b f",
                                  g=2, f=65)[:, :, :, 0:64],
                rinv[:])
            mean = mm[:, 0:C]
            var = spool.tile([D, C], f32, tag="var")
            nc.vector.scalar_tensor_tensor(var[:], mean, -1.0, mean,
                                           Alu.mult, Alu.mult)   # -mean^2
            nc.vector.tensor_add(var[:], var[:], mm[:, C:2 * C])
            epsb = spool.tile([D, 1], f32, tag="epsb")
            nc.vector.memset(epsb[:], EPS)
            # inv = rsqrt(var + eps)  (var+eps > 0, so |.| is a no-op)
            inv = spool.tile([D, C], f32, tag="inv")
            nc.scalar.activation(inv[:], var[:], Act.Abs_reciprocal_sqrt,
                                 bias=epsb[:])
            a_t = spool.tile([D, C], f32, tag="a_t")
            nc.vector.tensor_mul(a_t[:], inv[:], gam[:])
            negma = scrpool.tile([D, C], f32, tag="scr")
            nc.vector.scalar_tensor_tensor(negma[:], mean, -1.0, a_t[:],
                                           Alu.mult, Alu.mult)   # -mean*A
            b_t = spool.tile([D, C], f32, tag="b_t")
            nc.vector.tensor_add(b_t[:], bet[:], negma[:])

            # bf16 tables into rows 0:16 of the zero-padded K=128 tiles
            nc.vector.tensor_copy(tab_a[0:D, :], a_t[:])
            nc.scalar.activation(tab_b[0:D, :], b_t[:], Act.Copy)

            ohTv = ohT[:].rearrange("k (p i) -> k i p", i=CHUNKS)

            # ---- phase 2: gather A/B per row, Y = X*A + B ----
            with tc.tile_pool(name="ps2a", bufs=3, space="PSUM") as ps2a, \
                 tc.tile_pool(name="ps2b", bufs=3, space="PSUM") as ps2b:
                psa = [None] * CHUNKS
                psb = [None] * CHUNKS
                mt = [None] * CHUNKS
                yg = None

                def tail(i):
                    # add m into the B-gather bank, then evacuate Y (fp32)
                    nonlocal yg
                    nc.tensor.matmul(psb[i][:], ident[:], mt[i][:],
                                     start=False, stop=True)
                    g, k = divmod(i, GRP)
                    if k == 0:
                        yg = ypool.tile([P, GRP, C], f32, tag="yg",
                                        name="yg")
                    nc.scalar.activation(yg[:, k, :], psb[i][:], Act.Copy)
                    if k == GRP - 1:
                        nc.sync.dma_start(Yv[:, GRP * g:GRP * (g + 1), :],
                                          yg[:])

                for i in range(CHUNKS):
                    g, k = divmod(i, GRP)
                    psa[i] = ps2a.tile([P, C], f32, tag="psa", name="psa")
                    psb[i] = ps2b.tile([P, C], f32, tag="psb", name="psb")
                    nc.tensor.matmul(psa[i][:], ohTv[:, i, :], tab_a[:],
                                     start=True, stop=True)
                    nc.tensor.matmul(psb[i][:], ohTv[:, i, :], tab_b[:],
                                     start=True, stop=False)
                    if i >= STAG:
                        tail(i - STAG)
                    mt[i] = mpool.tile([P, C], bf16, tag="mt", name="mt")
                    nc.vector.tensor_mul(mt[i][:], xbs[g][:, k, :],
                                         psa[i][:])
                for i in range(CHUNKS - STAG, CHUNKS):
                    tail(i)

    nc.compile()
    return nc


def _get_program():
    if "nc" not in _CACHE:
        _CACHE["nc"] = _build_program()
    return _CACHE["nc"]


def kernel(X, d, parameter_t, fm_mean, gamma, beta):
    from concourse.bass_utils import run_bass_kernel_spmd

    X = np.ascontiguousarray(np.asarray(X), dtype=np.float32)
    d = np.ascontiguousarray(np.asarray(d), dtype=np.int32)
    gamma = np.ascontiguousarray(np.asarray(gamma), dtype=np.float32)
    beta = np.ascontiguousarray(np.asarray(beta), dtype=np.float32)

    nc = _get_program()
    in_maps = [
        {
            "X": X[c * SHARD:(c + 1) * SHARD],
            "d": d[c * SHARD:(c + 1) * SHARD],
            "gamma": gamma,
            "beta": beta,
        }
        for c in range(NCORES)
    ]
    res = run_bass_kernel_spmd(nc, in_maps, core_ids=list(range(NCORES)))
    out = np.concatenate([res.results[c]["Y"] for c in range(NCORES)], axis=0)
    return out.astype(np.float32, copy=False)
